# revision 14
# baseline (speedup 1.0000x reference)
"""ChildSumTreeLSTMCell on 8 Trainium2 NeuronCores.

Strategy: partition nodes across the 8 cores (bin-packed, not contiguous)
so every core's segment sums are fully local -- zero collectives.  Nodes are
bin-packed into tiles of <=128 nodes / <=512 edges (4 chunks of 128 edge
slots), giving ~98% slot utilization.  Segment sums are matmuls against a
0/1 membership matrix M4 built on the host and streamed as fp8e3 (exact for
0/1).  The forget-gate gather f[dst] * c_src factorizes to
f * segment_sum(c_src), removing the second scatter entirely.

v3: h/e/sf/df/M4 streams in fp8e3 (e3m4) to halve HBM traffic; c stays bf16
(direct path to the output).  The h*edge_w multiply reads edge_w straight
from PSUM (no evacuation).  Gate tail runs in bf16.  Output DMA rides the
vector queue so it cannot head-of-line-block input loads on sync.
"""

import sys

for _p in ("/opt/trn_rl_repo", "/root/.axon_site/_ro/trn_rl_repo"):
    if _p not in sys.path:
        sys.path.append(_p)

import numpy as np
import ml_dtypes

import concourse.bacc as bacc
import concourse.mybir as mybir
import concourse.tile as tile
from concourse.bass_utils import run_bass_kernel_spmd

F32 = mybir.dt.float32
BF16 = mybir.dt.bfloat16
F8E3 = mybir.dt.float8e3
F8E4 = mybir.dt.float8e4
U8 = mybir.dt.uint8

E = 500_000
N = 125_000
H = 128
G = 64
NCORES = 8
NPC = N // NCORES          # nodes per core
CHUNK = 128                # edges per chunk
CPT = 4                    # chunks per tile
TILE_E = CHUNK * CPT       # edge slots per tile
GRP = 4                    # node tiles per device loop group
bf16_np = ml_dtypes.bfloat16
f8e3_np = ml_dtypes.float8_e3m4

TRACE = False              # set by test.py to capture an NTFF profile
LAST = {}                  # last run's BassKernelResults


def _install_axon_hook():
    import types, contextlib, ctypes

    def _make_hook(so_path="/opt/axon/libaxon_pjrt.so"):
        lib = ctypes.CDLL(so_path)
        if not hasattr(lib, "axon_start_nrt_profile"):
            return None
        lib.axon_start_nrt_profile.argtypes = [
            ctypes.POINTER(ctypes.c_int64), ctypes.c_size_t]
        lib.axon_start_nrt_profile.restype = ctypes.c_int64
        lib.axon_stop_nrt_profile.argtypes = [ctypes.c_char_p]
        lib.axon_stop_nrt_profile.restype = ctypes.c_int64

        @contextlib.contextmanager
        def hook(output_dir, device_ids):
            import jax
            jax.devices()
            if device_ids:
                ids = (ctypes.c_int64 * len(device_ids))(*device_ids)
                rc = lib.axon_start_nrt_profile(ids, len(device_ids))
            else:
                rc = lib.axon_start_nrt_profile(None, 0)
            if rc != 0:
                raise RuntimeError("axon_start_nrt_profile rc=%d" % rc)
            try:
                yield
            finally:
                n = lib.axon_stop_nrt_profile(str(output_dir).encode())
                print("profile: %d file(s) written to %s" % (n, output_dir),
                      file=sys.stderr)

        return hook

    hook = _make_hook()
    mod = types.ModuleType("antenv.axon_hooks")
    mod.get_axon_ntff_profile_hook = lambda: hook
    mod.set_axon_ntff_profile_hook = lambda h: None
    sys.modules["antenv.axon_hooks"] = mod


def build_graph(T):
    """Build the per-core Bass graph for T node tiles."""
    nc = bacc.Bacc()
    dp = nc.declare_dram_parameter
    TEG = GRP * TILE_E
    NG = GRP * 128
    # blob bytes/partition: h fp8 (TEG) | e fp8 (TEG) | c bf16 (2*TEG) | m4 fp8e4 (TEG)
    BLOB = 5 * TEG
    blob_ext = dp("blob", [T // GRP, 128, BLOB], U8, isOutput=False)
    sfdf_ext = dp("sfdf", [T // GRP, G, 2 * TEG], F8E3, isOutput=False)
    oh4_ext = dp("oh4", [4, T * TILE_E], BF16, isOutput=False)
    welT_ext = dp("welT", [G, 128], F32, isOutput=False)
    wa_ext = dp("wa", [G, 4], F32, isOutput=False)
    bel_ext = dp("belB", [4, 128], F32, isOutput=False)
    wg_ext = {}
    for x in "fiuo":
        wg_ext[x] = dp("w%sT" % x, [2 * H, 128], F32, isOutput=False)
    bias_ext = {}
    for x in "fiuo":
        bias_ext[x] = (dp("bW%s" % x, [H], F32, isOutput=False),
                       dp("b%s" % x, [H], F32, isOutput=False))
    out_ext = dp("outT", [128, T * 2 * H], BF16, isOutput=True)

    with tile.TileContext(nc) as tc:
        cst = tc.alloc_tile_pool(name="cst", bufs=1)
        pin = tc.alloc_tile_pool(name="pin", bufs=6)
        pcv = tc.alloc_tile_pool(name="pcv", bufs=3)
        pnd = tc.alloc_tile_pool(name="pnd", bufs=2)
        pacc = tc.alloc_tile_pool(name="pacc", bufs=1, space="PSUM")
        pcsp = tc.alloc_tile_pool(name="pcsp", bufs=2, space="PSUM")
        pew = tc.alloc_tile_pool(name="pew", bufs=2, space="PSUM")
        pmm = tc.alloc_tile_pool(name="pmm", bufs=2, space="PSUM")

        # -- setup: constants -----------------------------------------------
        welT_sb = cst.tile([G, 128], F32)
        nc.sync.dma_start(out=welT_sb[:], in_=welT_ext[:])
        wa_sb = cst.tile([G, 4], F32)
        nc.sync.dma_start(out=wa_sb[:], in_=wa_ext[:])
        belr = cst.tile([4, 128], F32)
        nc.sync.dma_start(out=belr[:], in_=bel_ext[:])
        t2p = pmm.tile([4, 128], F32, tag="mm")
        nc.tensor.matmul(out=t2p[:], lhsT=wa_sb[:], rhs=welT_sb[:],
                         start=True, stop=True)
        wel_b16 = cst.tile([G, 128], BF16)
        nc.vector.tensor_copy(out=wel_b16[:], in_=welT_sb[:])
        t4b = cst.tile([4, 128], BF16)
        nc.vector.tensor_tensor(out=t4b[:], in0=t2p[:],
                                in1=belr[:], op=mybir.AluOpType.add)
        wtcomb = cst.tile([G + 4, 128], BF16)
        nc.sync.dma_start(out=wtcomb[0:G, :], in_=wel_b16[:])
        nc.sync.dma_start(out=wtcomb[G:G + 4, :], in_=t4b[:])

        wg = {}
        for x in "fiuo":
            stg = cst.tile([128, 128], F32, tag="wstg_%s" % x)
            nc.sync.dma_start(out=stg[:], in_=wg_ext[x][0:128, :])
            wa_t = cst.tile([128, 128], BF16, tag="wg_%s_a" % x)
            nc.vector.tensor_copy(out=wa_t[:], in_=stg[:])
            stg2 = cst.tile([128, 128], F32, tag="wstg2_%s" % x)
            nc.sync.dma_start(out=stg2[:], in_=wg_ext[x][128:256, :])
            wb_t = cst.tile([128, 128], BF16, tag="wg_%s_b" % x)
            nc.vector.tensor_copy(out=wb_t[:], in_=stg2[:])
            wg[x] = (wa_t, wb_t)

        bias = {}
        for x in "fiuo":
            b1 = cst.tile([128, 1], F32, tag="b1_%s" % x)
            nc.sync.dma_start(out=b1[:], in_=bias_ext[x][0][:, None])
            b2 = cst.tile([128, 1], F32, tag="b2_%s" % x)
            nc.sync.dma_start(out=b2[:], in_=bias_ext[x][1][:, None])
            bs = cst.tile([128, 1], F32, tag="bs_%s" % x)
            nc.vector.tensor_tensor(out=bs[:], in0=b1[:], in1=b2[:],
                                    op=mybir.AluOpType.add)
            bias[x] = bs

        # -- main loop: groups of GRP node tiles ----------------------------
        assert T % GRP == 0
        AF = mybir.ActivationFunctionType
        for g in range(T // GRP):
            blob = pin.tile([128, BLOB], U8, tag="blob")
            nc.sync.dma_start(out=blob[:], in_=blob_ext[g])
            heh = blob[:, 0:TEG].bitcast(F8E3)
            hee = blob[:, TEG:2 * TEG].bitcast(F8E3)
            cc = blob[:, 2 * TEG:4 * TEG].bitcast(BF16)
            m4 = blob[:, 4 * TEG:5 * TEG].bitcast(F8E4)
            sfdf = pin.tile([G, 2 * TEG], F8E3, tag="sfdf")
            nc.sync.dma_start(out=sfdf[:], in_=sfdf_ext[g])

            B68 = pcv.tile([G + 4, TEG], BF16, tag="B68")
            nc.sync.dma_start(
                out=B68[G:G + 4, :],
                in_=oh4_ext[:, g * TEG:(g + 1) * TEG])
            nc.vector.tensor_tensor(
                out=B68[0:G, :], in0=sfdf[:, 0:TEG], in1=sfdf[:, TEG:2 * TEG],
                op=mybir.AluOpType.mult)

            hs12 = pacc.tile([128, 2 * NG], F32, tag="hs12", space="PSUM")
            cs = pcsp.tile([128, NG], F32, tag="cs", space="PSUM")
            for tl in range(GRP):
                t0 = tl * TILE_E
                ew4 = pew.tile([128, TILE_E], F32, tag="ew4", space="PSUM")
                for c in range(CPT):
                    lsl = slice(t0 + c * CHUNK, t0 + (c + 1) * CHUNK)
                    nc.tensor.matmul(out=ew4[:, c * CHUNK:(c + 1) * CHUNK],
                                     lhsT=B68[:, lsl], rhs=wtcomb[:],
                                     start=True, stop=True)
                # e and c segment sums first (independent of hw4)
                for c in range(CPT):
                    lo = t0 + c * CHUNK
                    nc.tensor.matmul(
                        out=hs12[:, NG + tl * 128:NG + (tl + 1) * 128],
                        lhsT=hee[:, lo:lo + CHUNK],
                        rhs=m4[:, lo:lo + CHUNK],
                        start=(c == 0), stop=(c == CPT - 1))
                for c in range(CPT):
                    lo = t0 + c * CHUNK
                    nc.tensor.matmul(
                        out=cs[:, tl * 128:(tl + 1) * 128],
                        lhsT=cc[:, lo:lo + CHUNK],
                        rhs=m4[:, lo:lo + CHUNK],
                        start=(c == 0), stop=(c == CPT - 1))
                hw4 = pcv.tile([128, TILE_E], F8E4, tag="hw4")
                nc.vector.tensor_tensor(out=hw4[:], in0=heh[:, t0:t0 + TILE_E],
                                        in1=ew4[:], op=mybir.AluOpType.mult)
                hw4r = hw4[:].rearrange("p (k x) -> p k x", k=CPT)
                m4r = m4.rearrange("p (t x) -> p t x", t=GRP * CPT)
                for pr in range(CPT // 2):
                    nc.tensor.matmul(
                        out=hs12[:, tl * 128:(tl + 1) * 128],
                        lhsT=hw4r[:, 2 * pr:2 * pr + 2, :],
                        rhs=m4r[:, tl * CPT + 2 * pr:tl * CPT + 2 * pr + 2, :],
                        start=(pr == 0), stop=(pr == CPT // 2 - 1),
                        perf_mode=mybir.MatmulPerfMode.DoubleRow)

            hsab = pnd.tile([128, 2 * NG], BF16, tag="hsab")
            nc.scalar.activation(out=hsab[:], in_=hs12[:], func=AF.Copy)

            gate = {}
            for x, fn in (("f", "Sigmoid"), ("i", "Sigmoid"),
                          ("u", "Tanh"), ("o", "Sigmoid")):
                gp = pmm.tile([128, NG], F32, tag="mm", space="PSUM")
                nc.tensor.matmul(out=gp[:], lhsT=wg[x][0][:],
                                 rhs=hsab[:, 0:NG], start=True, stop=False)
                nc.tensor.matmul(out=gp[:], lhsT=wg[x][1][:],
                                 rhs=hsab[:, NG:2 * NG], start=False, stop=True)
                gs = pnd.tile([128, NG], BF16, tag="g_%s" % x)
                nc.scalar.activation(out=gs[:], in_=gp[:],
                                     func=getattr(AF, fn), bias=bias[x][:])
                gate[x] = gs

            # hc cols: [h x GRP | c x GRP]
            hc = pnd.tile([128, 2 * NG], BF16, tag="hc")
            ct = pnd.tile([128, NG], BF16, tag="ct")
            nc.vector.tensor_tensor(out=ct[:], in0=gate["f"][:], in1=cs[:],
                                    op=mybir.AluOpType.mult)
            iu = pnd.tile([128, NG], BF16, tag="iu")
            nc.gpsimd.tensor_tensor(out=iu[:], in0=gate["i"][:],
                                    in1=gate["u"][:], op=mybir.AluOpType.mult)
            nc.gpsimd.tensor_tensor(out=hc[:, NG:2 * NG], in0=iu[:],
                                    in1=ct[:], op=mybir.AluOpType.add)
            th = pnd.tile([128, NG], BF16, tag="th")
            nc.scalar.activation(out=th[:], in_=hc[:, NG:2 * NG], func=AF.Tanh)
            nc.gpsimd.tensor_tensor(out=hc[:, 0:NG], in0=gate["o"][:],
                                    in1=th[:], op=mybir.AluOpType.mult)
            nc.scalar.dma_start(
                out=out_ext[:, g * 2 * NG:(g + 1) * 2 * NG], in_=hc[:])

        for p in (pmm, pew, pcsp, pacc, pnd, pcv, pin, cst):
            p.release()
    nc.finalize()
    return nc


def plan_tiles(deg):
    """Bin-pack nodes into tiles of <=128 nodes and <=TILE_E edges.

    Two-pointer over degree-sorted nodes, keeping each tile's edge count
    tracking the target density so node and edge budgets fill together.
    Returns a list of node-index arrays.
    """
    order = np.argsort(-deg, kind="stable")
    ds = deg[order]
    f, b = 0, len(order) - 1
    tiles = []
    while f <= b:
        nodes = []
        e = 0
        while f <= b and len(nodes) < 128:
            n = len(nodes)
            if e * 128 <= TILE_E * n or f == b:   # below target -> dense
                if e + ds[f] <= TILE_E:
                    nodes.append(order[f]); e += ds[f]; f += 1
                elif e + ds[b] <= TILE_E:
                    nodes.append(order[b]); e += ds[b]; b -= 1
                else:
                    break
            else:
                if e + ds[b] <= TILE_E:
                    nodes.append(order[b]); e += ds[b]; b -= 1
                elif e + ds[f] <= TILE_E:
                    nodes.append(order[f]); e += ds[f]; f += 1
                else:
                    break
        tiles.append(np.array(nodes, dtype=np.int64))
    return tiles


def prep_core(k, h_src, c_src, embed_dst, src_f, dst_f, etype, dst, T):
    """Build one core's padded, tiled input arrays + output unmap columns."""
    lo = k * NPC
    sel = np.nonzero((dst >= lo) & (dst < lo + NPC))[0]
    dl = (dst[sel] - lo).astype(np.int64)
    deg = np.bincount(dl, minlength=NPC)
    tiles = plan_tiles(deg)
    Tk = len(tiles)
    assert Tk <= T
    ES = T * TILE_E

    tile_of = np.empty(NPC, np.int64)
    local_of = np.empty(NPC, np.int64)
    for t, nodes in enumerate(tiles):
        tile_of[nodes] = t
        local_of[nodes] = np.arange(len(nodes))

    et = tile_of[dl]
    order2 = np.argsort(et, kind="stable")
    cnt = np.bincount(et, minlength=T)
    off = np.concatenate([[0], np.cumsum(cnt)])
    slot_in_tile = np.arange(len(dl)) - off[et[order2]]
    slots = et[order2] * TILE_E + slot_in_tile
    eidx = sel[order2]          # global edge index per slot
    lids = local_of[dl[order2]]  # local node id per slot

    val = np.zeros(ES, dtype=bool)
    val[slots] = True
    gi = np.empty(ES, dtype=np.int64)
    gi[slots] = eidx

    def pad_rows(a, w):
        out = np.zeros((ES, w), dtype=np.float32)
        out[val] = a[gi[val]]
        return out

    def chunk_layout(a):
        # [ES, W] -> [T, 128, CPT*W] with slot (c*128+p) at [t, p, c*W:...]
        W = a.shape[1]
        return np.ascontiguousarray(
            a.reshape(T, CPT, CHUNK, W).transpose(0, 2, 1, 3)
             .reshape(T, 128, CPT * W))

    def pair(a):
        # [T,P,W] -> [T//GRP,P,GRP*W] grouping consecutive tiles
        Tn, P, W = a.shape
        return a.reshape(Tn // GRP, GRP, P, W).transpose(0, 2, 1, 3) \
                .reshape(Tn // GRP, P, GRP * W)

    hp = pair(chunk_layout(pad_rows(h_src, H))).astype(f8e3_np)
    ep = pair(chunk_layout(pad_rows(embed_dst, H))).astype(f8e3_np)
    ccp = pair(chunk_layout(pad_rows(c_src, H))).astype(bf16_np)
    # membership one-hot: 0x38 is fp8e4 1.0
    A = np.zeros((ES, 128), dtype=np.uint8)
    A[slots, lids] = 0x38
    m4 = pair(chunk_layout(A))
    blob = np.ascontiguousarray(np.concatenate(
        [hp.view(np.uint8), ep.view(np.uint8), ccp.view(np.uint8), m4],
        axis=2))
    sf = pair(pad_rows(src_f, G).reshape(T, TILE_E, G).transpose(0, 2, 1))
    df = pair(pad_rows(dst_f, G).reshape(T, TILE_E, G).transpose(0, 2, 1))
    sfdf = np.ascontiguousarray(
        np.concatenate([sf, df], axis=2)).astype(f8e3_np)
    oh = np.zeros((ES, 4), dtype=np.float32)
    oh[slots, etype[eidx]] = 1.0
    oh[slots, 3] = 1.0
    ohT = np.ascontiguousarray(oh.T).astype(bf16_np)

    # output unmap: per local node, the column of its h (c at +NG)
    NG = GRP * 128
    gcol = (tile_of // GRP) * 2 * NG + (tile_of % GRP) * 128 + local_of
    return ({"blob": blob, "sfdf": sfdf, "oh4": ohT}, gcol)


def _belB(b_el):
    out = np.zeros((4, 128), dtype=np.float32)
    out[3] = b_el
    return out


_graph_cache = {}


def kernel(**inputs):
    h_src = np.asarray(inputs["h_src"], dtype=np.float32)
    c_src = np.asarray(inputs["c_src"], dtype=np.float32)
    embed_dst = np.asarray(inputs["embed_dst"], dtype=np.float32)
    src_f = np.asarray(inputs["src_node_feat"], dtype=np.float32)
    dst_f = np.asarray(inputs["dst_node_feat"], dtype=np.float32)
    etype = np.asarray(inputs["edge_type_idx"]).astype(np.int64)
    dst = np.asarray(inputs["dst_idx"]).astype(np.int64)

    weights = {
        "welT": np.ascontiguousarray(np.asarray(inputs["W_el"], np.float32).T),
        "wa": np.ascontiguousarray(np.concatenate(
            [np.asarray(inputs["W_eoh"], np.float32),
             np.asarray(inputs["b_eoh"], np.float32)[:, None]], axis=1)),
        "belB": _belB(np.asarray(inputs["b_el"], np.float32)),
    }
    for x, wn, bwn, bn in (("f", "Wf", "bWf", "bf"), ("i", "Wi", "bWi", "bi"),
                           ("u", "Wu", "bWu", "bu"), ("o", "Wo", "bWo", "bo")):
        weights["w%sT" % x] = np.ascontiguousarray(
            np.asarray(inputs[wn], np.float32).T)
        weights["bW%s" % x] = np.asarray(inputs[bwn], np.float32)
        weights["b%s" % x] = np.asarray(inputs[bn], np.float32)

    planned = []
    for k in range(NCORES):
        lo = k * NPC
        m = (dst >= lo) & (dst < lo + NPC)
        deg = np.bincount(dst[m] - lo, minlength=NPC)
        planned.append(len(plan_tiles(deg)))
    T = max(planned)
    T += (-T) % GRP  # grouped tiling needs T % GRP == 0

    in_maps = []
    gcols = []
    for k in range(NCORES):
        m, gcol = prep_core(k, h_src, c_src, embed_dst, src_f, dst_f,
                            etype, dst, T)
        m.update(weights)
        in_maps.append(m)
        gcols.append(gcol)

    if T not in _graph_cache:
        _graph_cache[T] = build_graph(T)
    nc = _graph_cache[T]

    if TRACE:
        _install_axon_hook()
    res = run_bass_kernel_spmd(nc, in_maps, list(range(NCORES)), trace=TRACE)
    LAST["res"] = res

    NG = GRP * 128
    out = np.empty((N, 2 * H), dtype=np.float32)
    for k in range(NCORES):
        outT = np.asarray(res.results[k]["outT"]).astype(np.float32)
        gcol = gcols[k]
        base = k * NPC
        out[base:base + NPC, 0:H] = outT[:, gcol].T
        out[base:base + NPC, H:2 * H] = outT[:, gcol + NG].T
    return out
